# revision 16
# baseline (speedup 1.0000x reference)
"""Trainium2 Bass kernel for CrossAttentionConditionInjection.

Math: the attention keys/values come from a single condition token broadcast
across the sequence, so the scores are constant along the key axis; softmax is
exactly uniform and the attention output collapses to

    out[b, s, :] = (condition[b] @ Wv.T + bv) @ Wo.T + bo      (for every s)

independent of hidden_states / Wq / Wk / q entirely.

Sharding: core i owns output columns [128*i, 128*(i+1)) for BOTH batches.
Weights stream in bf16 (rel-err budget 2e-2 >> bf16's ~4e-3).  All inputs ride
ONE merged bf16 tensor split into ~256KB chunks spread over the THREE
DMA-capable queues (sync/scalar HWDGE + gpsimd SWDGE) so the combined read
rate saturates the per-core HBM limit; stage 1 consumes j-chunks in expected
arrival order.  smalls packs bv (f32 bit-punned), condT and bo and is
recovered with AP bitcasts.  Compute is all PE:

  stage 1: per j-chunk jc, 8 matmuls  lhsT=WvT block [128k,128j] (stationary),
           rhs=condT k-slice [128k, 2b] -> v1T_ps[.., 2]  (accum over k),
           double-buffered across two PSUM banks so the DVE bv-add/cast of
           chunk jc never blocks the matmuls of chunk jc+1 (PSUM WAR is
           tracked at bank granularity).
  stage 2: per batch b: 8 matmuls lhsT=v1T_sb jc-slice [128j, 1],
           rhs=WoT block [128j, 128n] -> row_b [1, 128n] (+ [1,1]x[1,128]
           ones-matmul folds bo), then one matmul lhsT=ones[1,128],
           rhs=row_b bf16 -> out_b [128 (s-copies), 128n].
  write:   batch 0 as one 1MB DMA on sync, batch 1 as two 512KB halves on
           scalar+gpsimd; each broadcast-writes its [128,128] row tile with
           per-partition-contiguous lines.  Batch-0's write overlaps batch-1's
           tail compute.

A few dummy matmuls head the PE queue to lift the HAM clock gate
(1.2 -> 2.4 GHz) while the first weight chunks are still in flight.
"""

import numpy as np
from contextlib import ExitStack

import ml_dtypes

import concourse.bass as bass
import concourse.bacc as bacc
import concourse.mybir as mybir
import concourse.tile as tile
from concourse.bass_utils import run_bass_kernel_spmd

B, S, D = 2, 2048, 1024
NCORES = 8
NW = D // NCORES  # 128 output columns per core
KC = D // 128  # 8 contraction chunks (k)
JC = D // 128  # 8 v1 chunks (j)
SA = S // 128  # 16 sequence blocks
BF16 = ml_dtypes.bfloat16

SM = 160  # smalls section cols (bf16): 16 bvT-f32-punned, 16 condT, 128 bo
# bf16 column offset of each jc block / the wo block inside the merged tensor
_COL = {0: SM, 1: SM + 1024, 2: SM + 2048, 3: SM + 3072, "wo": SM + 4096,
        4: SM + 5120, 5: SM + 6144, 6: SM + 7168, 7: SM + 8192}
WVO_COLS = SM + 9 * 1024  # 9376
# input chunks spread over the three DMA queues, ~balanced bytes each
_CHUNKS = [  # (engine, start, end)
    ("sync", 0, _COL[1]),                    # smalls + jc0      (296 KB)
    ("scalar", _COL["wo"], _COL[4]),         # wo                (256 KB)
    ("gpsimd", _COL[2], _COL[3]),            # jc2               (256 KB)
    ("sync", _COL[3], _COL["wo"]),           # jc3               (256 KB)
    ("scalar", _COL[1], _COL[2]),            # jc1               (256 KB)
    ("gpsimd", _COL[5], _COL[6]),            # jc5               (256 KB)
    ("sync", _COL[6], _COL[7]),              # jc6               (256 KB)
    ("scalar", _COL[4], _COL[5]),            # jc4               (256 KB)
    ("gpsimd", _COL[7], WVO_COLS),           # jc7               (256 KB)
]
# stage-1 consumption order ~ expected chunk arrival order
_ORDER = [0, 2, 1, 3, 5, 4, 6, 7]

_cache = {}


def _build():
    f32 = mybir.dt.float32
    bf16 = mybir.dt.bfloat16
    nc = bacc.Bacc()

    wvo = nc.dram_tensor("wvo", [128, WVO_COLS], bf16, kind="ExternalInput")
    y = nc.dram_tensor("y", [128, B * SA * NW], f32, kind="ExternalOutput")

    with tile.TileContext(nc) as tc, ExitStack() as ctx:
        wvo_pool = ctx.enter_context(tc.tile_pool(name="wvo", bufs=1))
        small = ctx.enter_context(tc.tile_pool(name="small", bufs=1))
        outp = ctx.enter_context(tc.tile_pool(name="outp", bufs=1))
        pools = {}
        for name in ("warm", "v1a", "v1b", "out0", "out1"):
            pools[name] = ctx.enter_context(
                tc.tile_pool(name=f"ps_{name}", bufs=1, space=bass.MemorySpace.PSUM)
            )

        from concourse.tile_rust import add_dep_helper

        # constants (no DMA needed)
        ones1 = small.tile([1, NW], bf16)
        nc.vector.memset(ones1[:], 1.0)
        warm = small.tile([128, 128], bf16)
        nc.vector.memset(warm[:], 0.0)

        # ---- loads: chunks spread across the 3 DMA queues ----
        wvo_sb = wvo_pool.tile([128, WVO_COLS], bf16)
        engines = {"sync": nc.sync, "scalar": nc.scalar, "gpsimd": nc.gpsimd}
        prev = {k: None for k in engines}
        for ring, c0, c1 in _CHUNKS:
            d = engines[ring].dma_start(wvo_sb[:, c0:c1], wvo[:, c0:c1])
            if prev[ring] is not None:
                add_dep_helper(d.ins, prev[ring].ins, sync=False, reason="ring order")
            prev[ring] = d

        bvT = wvo_sb[:, 0:16].bitcast(f32)  # [128, 8] f32
        condT = wvo_sb[:, 16:32]  # [128, 16] bf16
        bo_ap = wvo_sb[0:1, 32:160]  # [1, 128] bf16

        # ---- PE warmup: lift the HAM clock gate while DMA streams in ----
        warm_ps = pools["warm"].tile([128, 128], f32)
        for _ in range(10):
            nc.tensor.matmul(warm_ps[:], warm[:], warm[:], start=True, stop=True)

        # ---- stage 1: v1T[:, jc*2+b] = sum_k WvT[k, jc*128+p] cond[b, k] ----
        # the DVE then folds bv + casts + broadcasts each v1T column across
        # 128 free columns (v1bc) so stage 2's matmul lands the output row
        # already replicated across all 128 partitions (seq positions).
        v1_ps = [pools["v1a"].tile([128, JC], f32, name="v1a_t"),
                 pools["v1b"].tile([128, JC], f32, name="v1b_t")]
        v1bc = small.tile([128, JC * B * 128], bf16)
        out_ps = [pools["out0"].tile([128, NW], f32, name="out0_t"),
                  pools["out1"].tile([128, NW], f32, name="out1_t")]
        wo0 = _COL["wo"]
        for n, jc in enumerate(_ORDER):
            base = _COL[jc]
            acc = v1_ps[n % 2][:, (n // 2) * B : (n // 2) * B + B]
            for kc in range(KC):
                nc.tensor.matmul(
                    acc,
                    wvo_sb[:, base + kc * 128 : base + (kc + 1) * 128],
                    condT[:, kc * B : (kc + 1) * B],
                    start=(kc == 0),
                    stop=(kc == KC - 1),
                )
            for b in range(B):
                nc.vector.tensor_add(
                    v1bc[:, (jc * B + b) * 128 : (jc * B + b + 1) * 128],
                    acc[:, b : b + 1].broadcast_to([128, 128]),
                    bvT[:, jc : jc + 1].broadcast_to([128, 128]),
                )
            # stage 2: out_b[m, n] += sum_j v1bc[j, m] WoT[j, n]  (all m equal)
            for b in range(B):
                nc.tensor.matmul(
                    out_ps[b][:],
                    v1bc[:, (jc * B + b) * 128 : (jc * B + b + 1) * 128],
                    wvo_sb[:, wo0 + jc * NW : wo0 + (jc + 1) * NW],
                    start=(n == 0),
                    stop=False,
                    skip_group_check=True,
                )
        out_sb = outp.tile([128, B * NW], f32)
        for b in range(B):
            nc.tensor.matmul(
                out_ps[b][:], ones1[:], bo_ap, start=False, stop=True,
                skip_group_check=True,
            )
            nc.vector.tensor_copy(out_sb[:, b * NW : (b + 1) * NW], out_ps[b][:])

        # ---- broadcast-writes, ~thirds across the 3 queues ----
        def wr(eng_name, b, a0, a1):
            eng = engines[eng_name]
            d = eng.dma_start(
                y[:, b * SA * NW + a0 * NW : b * SA * NW + a1 * NW].rearrange(
                    "p (a c) -> p a c", a=a1 - a0
                ),
                out_sb[:, b * NW : (b + 1) * NW][:, None, :].broadcast_to(
                    [128, a1 - a0, NW]
                ),
            )
            add_dep_helper(d.ins, prev[eng_name].ins, sync=False, reason="ring order")

        wr("sync", 0, 0, 13)
        wr("scalar", 0, 13, SA)
        wr("scalar", 1, 0, 9)
        wr("gpsimd", 1, 9, SA)

    nc.compile()
    return nc


def _prep_inputs(condition, Wv, bv, Wo, bo):
    cond = np.asarray(condition, np.float32)
    Wv = np.asarray(Wv, np.float32)
    Wo = np.asarray(Wo, np.float32)
    bv = np.asarray(bv, np.float32)
    bo = np.asarray(bo, np.float32)

    # wv block jc: [p, kc*128+c] = Wv[jc*128+c, kc*128+p]
    wv_blocks = (
        Wv.reshape(JC, 128, KC, 128).transpose(3, 0, 2, 1).astype(BF16)
    )  # [128, JC, KC, 128]
    # condT[p, kc*B+b] = cond[b, kc*128+p]
    condT = np.ascontiguousarray(
        cond.T.reshape(KC, 128, B).transpose(1, 0, 2).reshape(128, KC * B)
    ).astype(BF16)
    bvT = np.ascontiguousarray(bv.reshape(JC, 128).T)  # [128, 8] f32

    smalls_base = np.zeros((128, SM), BF16)
    smalls_base[:, 0:16] = bvT.view(BF16)  # bit-punned f32
    smalls_base[:, 16:32] = condT

    in_maps = []
    for i in range(NCORES):
        # wo[p, jc*NW+c] = Wo[i*NW+c, jc*128+p]
        wo_i = np.ascontiguousarray(
            Wo[i * NW : (i + 1) * NW]
            .reshape(NW, JC, 128)
            .transpose(2, 1, 0)
            .reshape(128, JC * NW)
        ).astype(BF16)
        smalls = smalls_base.copy()
        smalls[0, 32:160] = bo[i * NW : (i + 1) * NW].astype(BF16)
        wvo = np.concatenate(
            [
                smalls,
                wv_blocks[:, 0:2].reshape(128, 2048),
                wv_blocks[:, 2:4].reshape(128, 2048),
                wo_i,
                wv_blocks[:, 4:6].reshape(128, 2048),
                wv_blocks[:, 6:8].reshape(128, 2048),
            ],
            axis=1,
        )
        in_maps.append({"wvo": np.ascontiguousarray(wvo)})
    return in_maps


def _run(in_maps, **kwargs):
    if "nc" not in _cache:
        _cache["nc"] = _build()
    return run_bass_kernel_spmd(
        _cache["nc"], in_maps, core_ids=list(range(NCORES)), **kwargs
    )


def kernel(hidden_states, condition, Wq, bq, Wk, bk, Wv, bv, Wo, bo):
    in_maps = _prep_inputs(condition, Wv, bv, Wo, bo)
    res = _run(in_maps)
    full = np.empty((B, S, D), np.float32)
    for i in range(NCORES):
        yv = np.asarray(res.results[i]["y"]).reshape(128, B, SA, NW)
        full[:, :, i * NW : (i + 1) * NW] = (
            yv.transpose(1, 2, 0, 3).reshape(B, S, NW)
        )
    return full


# revision 17
# speedup vs baseline: 1.1507x; 1.1507x over previous
"""Trainium2 Bass kernel for CrossAttentionConditionInjection.

Math: the attention keys/values come from a single condition token broadcast
across the sequence, so the scores are constant along the key axis; softmax is
exactly uniform and the attention output collapses to

    out[b, s, :] = (condition[b] @ Wv.T + bv) @ Wo.T + bo      (for every s)

independent of hidden_states / Wq / Wk / q entirely.

Sharding: core i owns output columns [128*i, 128*(i+1)) for BOTH batches.
Weights stream in bf16 (rel-err budget 2e-2 >> bf16's ~4e-3).  All inputs ride
ONE merged bf16 tensor split into ~256KB chunks spread over the THREE
DMA-capable queues (sync/scalar HWDGE + gpsimd SWDGE) so the combined read
rate saturates the per-core HBM limit; stage 1 consumes j-chunks in expected
arrival order.  smalls packs bv (f32 bit-punned), condT and bo and is
recovered with AP bitcasts.  Compute is all PE:

  stage 1: per j-chunk jc, 8 matmuls  lhsT=WvT block [128k,128j] (stationary),
           rhs=condT k-slice [128k, 2b] -> v1T_ps[.., 2]  (accum over k),
           double-buffered across two PSUM banks so the DVE bv-add/cast of
           chunk jc never blocks the matmuls of chunk jc+1 (PSUM WAR is
           tracked at bank granularity).
  stage 2: per batch b: 8 matmuls lhsT=v1T_sb jc-slice [128j, 1],
           rhs=WoT block [128j, 128n] -> row_b [1, 128n] (+ [1,1]x[1,128]
           ones-matmul folds bo), then one matmul lhsT=ones[1,128],
           rhs=row_b bf16 -> out_b [128 (s-copies), 128n].
  write:   batch 0 as one 1MB DMA on sync, batch 1 as two 512KB halves on
           scalar+gpsimd; each broadcast-writes its [128,128] row tile with
           per-partition-contiguous lines.  Batch-0's write overlaps batch-1's
           tail compute.

A few dummy matmuls head the PE queue to lift the HAM clock gate
(1.2 -> 2.4 GHz) while the first weight chunks are still in flight.
"""

import numpy as np
from contextlib import ExitStack

import ml_dtypes

import concourse.bass as bass
import concourse.bacc as bacc
import concourse.mybir as mybir
import concourse.tile as tile
from concourse.bass_utils import run_bass_kernel_spmd

B, S, D = 2, 2048, 1024
NCORES = 8
NW = D // NCORES  # 128 output columns per core
KC = D // 128  # 8 contraction chunks (k)
JC = D // 128  # 8 v1 chunks (j)
SA = S // 128  # 16 sequence blocks
BF16 = ml_dtypes.bfloat16

SM = 160  # smalls section cols (bf16): 16 bvT-f32-punned, 16 condT, 128 bo
# bf16 column offset of each jc block / the wo block inside the merged tensor
_COL = {0: SM, 1: SM + 1024, 2: SM + 2048, 3: SM + 3072, "wo": SM + 4096,
        4: SM + 5120, 5: SM + 6144, 6: SM + 7168, 7: SM + 8192}
WVO_COLS = SM + 9 * 1024  # 9376
# input chunks spread over the three DMA queues, ~balanced bytes each
_CHUNKS = [  # (engine, start, end)
    ("sync", 0, _COL[1]),                    # smalls + jc0      (296 KB)
    ("scalar", _COL["wo"], _COL[4]),         # wo                (256 KB)
    ("gpsimd", _COL[2], _COL[3]),            # jc2               (256 KB)
    ("sync", _COL[3], _COL["wo"]),           # jc3               (256 KB)
    ("scalar", _COL[1], _COL[2]),            # jc1               (256 KB)
    ("gpsimd", _COL[5], _COL[6]),            # jc5               (256 KB)
    ("sync", _COL[6], _COL[7]),              # jc6               (256 KB)
    ("scalar", _COL[4], _COL[5]),            # jc4               (256 KB)
    ("gpsimd", _COL[7], WVO_COLS),           # jc7               (256 KB)
]
# stage-1 consumption order ~ expected chunk arrival order
_ORDER = [0, 2, 1, 3, 5, 4, 6, 7]

_cache = {}


def _build():
    f32 = mybir.dt.float32
    bf16 = mybir.dt.bfloat16
    nc = bacc.Bacc()

    wvo = nc.dram_tensor("wvo", [128, WVO_COLS], bf16, kind="ExternalInput")
    y = nc.dram_tensor("y", [128, B * SA * NW], bf16, kind="ExternalOutput")

    with tile.TileContext(nc) as tc, ExitStack() as ctx:
        wvo_pool = ctx.enter_context(tc.tile_pool(name="wvo", bufs=1))
        small = ctx.enter_context(tc.tile_pool(name="small", bufs=1))
        outp = ctx.enter_context(tc.tile_pool(name="outp", bufs=1))
        pools = {}
        for name in ("warm", "v1a", "v1b", "row0", "row1", "out0", "out1"):
            pools[name] = ctx.enter_context(
                tc.tile_pool(name=f"ps_{name}", bufs=1, space=bass.MemorySpace.PSUM)
            )

        from concourse.tile_rust import add_dep_helper

        # constants (no DMA needed)
        ones1 = small.tile([1, NW], bf16)
        nc.vector.memset(ones1[:], 1.0)
        warm = small.tile([128, 128], bf16)
        nc.vector.memset(warm[:], 0.0)

        # ---- loads: chunks spread across the 3 DMA queues ----
        wvo_sb = wvo_pool.tile([128, WVO_COLS], bf16)
        engines = {"sync": nc.sync, "scalar": nc.scalar, "gpsimd": nc.gpsimd}
        prev = {k: None for k in engines}
        for ring, c0, c1 in _CHUNKS:
            d = engines[ring].dma_start(wvo_sb[:, c0:c1], wvo[:, c0:c1])
            if prev[ring] is not None:
                add_dep_helper(d.ins, prev[ring].ins, sync=False, reason="ring order")
            prev[ring] = d

        bvT = wvo_sb[:, 0:16].bitcast(f32)  # [128, 8] f32
        condT = wvo_sb[:, 16:32]  # [128, 16] bf16
        bo_ap = wvo_sb[0:1, 32:160]  # [1, 128] bf16

        # ---- PE warmup: lift the HAM clock gate while DMA streams in ----
        warm_ps = pools["warm"].tile([128, 128], f32)
        for _ in range(10):
            nc.tensor.matmul(warm_ps[:], warm[:], warm[:], start=True, stop=True)

        # ---- stage 1: v1T[:, jc*2+b] = sum_k WvT[k, jc*128+p] cond[b, k] ----
        v1_ps = [pools["v1a"].tile([128, JC], f32, name="v1a_t"),
                 pools["v1b"].tile([128, JC], f32, name="v1b_t")]
        v1T_sb = small.tile([128, JC * B], bf16)
        for n, jc in enumerate(_ORDER):
            base = _COL[jc]
            acc = v1_ps[n % 2][:, (n // 2) * B : (n // 2) * B + B]
            for kc in range(KC):
                nc.tensor.matmul(
                    acc,
                    wvo_sb[:, base + kc * 128 : base + (kc + 1) * 128],
                    condT[:, kc * B : (kc + 1) * B],
                    start=(kc == 0),
                    stop=(kc == KC - 1),
                )
            # fold in bv, cast to bf16 for stage 2
            nc.vector.tensor_add(
                v1T_sb[:, jc * B : (jc + 1) * B],
                acc,
                bvT[:, jc : jc + 1].broadcast_to([128, B]),
            )

        # ---- stage 2 per batch: row[b, n] = sum_j v1[b, j] WoT[j, n] + bo ----
        wo0 = _COL["wo"]
        row_ps = [pools["row0"].tile([1, NW], f32, name="row0_t"),
                  pools["row1"].tile([1, NW], f32, name="row1_t")]
        row_sb = [small.tile([1, NW], bf16, name=f"row_sb{b}") for b in range(B)]
        out_ps = [pools["out0"].tile([128, NW], f32, name="out0_t"),
                  pools["out1"].tile([128, NW], f32, name="out1_t")]
        out_sb = outp.tile([128, B * NW], bf16)
        for b in range(B):
            for i, jc in enumerate(_ORDER):
                nc.tensor.matmul(
                    row_ps[b][:],
                    v1T_sb[:, jc * B + b : jc * B + b + 1],
                    wvo_sb[:, wo0 + jc * NW : wo0 + (jc + 1) * NW],
                    start=(i == 0),
                    stop=False,
                )
            nc.tensor.matmul(
                row_ps[b][:], ones1[0:1, 0:1], bo_ap, start=False, stop=True
            )
            nc.vector.tensor_copy(row_sb[b][:], row_ps[b][:])
            # broadcast across partitions (seq positions)
            nc.tensor.matmul(
                out_ps[b][:], ones1[:], row_sb[b][:], start=True, stop=True
            )
            nc.vector.tensor_copy(out_sb[:, b * NW : (b + 1) * NW], out_ps[b][:])

        # ---- broadcast-writes (bf16), one batch per HWDGE queue ----
        def wr(eng_name, b, a0, a1):
            eng = engines[eng_name]
            d = eng.dma_start(
                y[:, b * SA * NW + a0 * NW : b * SA * NW + a1 * NW].rearrange(
                    "p (a c) -> p a c", a=a1 - a0
                ),
                out_sb[:, b * NW : (b + 1) * NW][:, None, :].broadcast_to(
                    [128, a1 - a0, NW]
                ),
            )
            add_dep_helper(d.ins, prev[eng_name].ins, sync=False, reason="ring order")

        wr("sync", 0, 0, SA)
        wr("scalar", 1, 0, SA)

    nc.compile()
    return nc


def _prep_inputs(condition, Wv, bv, Wo, bo):
    cond = np.asarray(condition, np.float32)
    Wv = np.asarray(Wv, np.float32)
    Wo = np.asarray(Wo, np.float32)
    bv = np.asarray(bv, np.float32)
    bo = np.asarray(bo, np.float32)

    # wv block jc: [p, kc*128+c] = Wv[jc*128+c, kc*128+p]
    wv_blocks = (
        Wv.reshape(JC, 128, KC, 128).transpose(3, 0, 2, 1).astype(BF16)
    )  # [128, JC, KC, 128]
    # condT[p, kc*B+b] = cond[b, kc*128+p]
    condT = np.ascontiguousarray(
        cond.T.reshape(KC, 128, B).transpose(1, 0, 2).reshape(128, KC * B)
    ).astype(BF16)
    bvT = np.ascontiguousarray(bv.reshape(JC, 128).T)  # [128, 8] f32

    smalls_base = np.zeros((128, SM), BF16)
    smalls_base[:, 0:16] = bvT.view(BF16)  # bit-punned f32
    smalls_base[:, 16:32] = condT

    in_maps = []
    for i in range(NCORES):
        # wo[p, jc*NW+c] = Wo[i*NW+c, jc*128+p]
        wo_i = np.ascontiguousarray(
            Wo[i * NW : (i + 1) * NW]
            .reshape(NW, JC, 128)
            .transpose(2, 1, 0)
            .reshape(128, JC * NW)
        ).astype(BF16)
        smalls = smalls_base.copy()
        smalls[0, 32:160] = bo[i * NW : (i + 1) * NW].astype(BF16)
        wvo = np.concatenate(
            [
                smalls,
                wv_blocks[:, 0:2].reshape(128, 2048),
                wv_blocks[:, 2:4].reshape(128, 2048),
                wo_i,
                wv_blocks[:, 4:6].reshape(128, 2048),
                wv_blocks[:, 6:8].reshape(128, 2048),
            ],
            axis=1,
        )
        in_maps.append({"wvo": np.ascontiguousarray(wvo)})
    return in_maps


def _run(in_maps, **kwargs):
    if "nc" not in _cache:
        _cache["nc"] = _build()
    return run_bass_kernel_spmd(
        _cache["nc"], in_maps, core_ids=list(range(NCORES)), **kwargs
    )


def kernel(hidden_states, condition, Wq, bq, Wk, bk, Wv, bv, Wo, bo):
    in_maps = _prep_inputs(condition, Wv, bv, Wo, bo)
    res = _run(in_maps)
    full = np.empty((B, S, D), np.float32)
    for i in range(NCORES):
        yv = np.asarray(res.results[i]["y"]).astype(np.float32).reshape(128, B, SA, NW)
        full[:, :, i * NW : (i + 1) * NW] = (
            yv.transpose(1, 2, 0, 3).reshape(B, S, NW)
        )
    return full


# revision 18
# speedup vs baseline: 1.1536x; 1.0025x over previous
"""Trainium2 Bass kernel for CrossAttentionConditionInjection.

Math: the attention keys/values come from a single condition token broadcast
across the sequence, so the scores are constant along the key axis; softmax is
exactly uniform and the attention output collapses to

    out[b, s, :] = (condition[b] @ Wv.T + bv) @ Wo.T + bo      (for every s)

independent of hidden_states / Wq / Wk / q entirely.

Sharding: core i owns output columns [128*i, 128*(i+1)) for BOTH batches.
Weights stream in bf16 (rel-err budget 2e-2 >> bf16's ~4e-3).  All inputs ride
ONE merged bf16 tensor split into ~256KB chunks spread over the THREE
DMA-capable queues (sync/scalar HWDGE + gpsimd SWDGE) so the combined read
rate saturates the per-core HBM limit; stage 1 consumes j-chunks in expected
arrival order.  smalls packs bv (f32 bit-punned), condT and bo and is
recovered with AP bitcasts.  Compute is all PE:

  stage 1: per j-chunk jc, 8 matmuls  lhsT=WvT block [128k,128j] (stationary),
           rhs=condT k-slice [128k, 2b] -> v1T_ps[.., 2]  (accum over k),
           double-buffered across two PSUM banks so the DVE bv-add/cast of
           chunk jc never blocks the matmuls of chunk jc+1 (PSUM WAR is
           tracked at bank granularity).
  stage 2: per batch b: 8 matmuls lhsT=v1T_sb jc-slice [128j, 1],
           rhs=WoT block [128j, 128n] -> row_b [1, 128n] (+ [1,1]x[1,128]
           ones-matmul folds bo), then one matmul lhsT=ones[1,128],
           rhs=row_b bf16 -> out_b [128 (s-copies), 128n].
  write:   batch 0 as one 1MB DMA on sync, batch 1 as two 512KB halves on
           scalar+gpsimd; each broadcast-writes its [128,128] row tile with
           per-partition-contiguous lines.  Batch-0's write overlaps batch-1's
           tail compute.

A few dummy matmuls head the PE queue to lift the HAM clock gate
(1.2 -> 2.4 GHz) while the first weight chunks are still in flight.
"""

import numpy as np
from contextlib import ExitStack

import ml_dtypes

import concourse.bass as bass
import concourse.bacc as bacc
import concourse.mybir as mybir
import concourse.tile as tile
from concourse.bass_utils import run_bass_kernel_spmd

B, S, D = 2, 2048, 1024
NCORES = 8
NW = D // NCORES  # 128 output columns per core
KC = D // 128  # 8 contraction chunks (k)
JC = D // 128  # 8 v1 chunks (j)
SA = S // 128  # 16 sequence blocks
BF16 = ml_dtypes.bfloat16

SM = 160  # smalls section cols (bf16): 16 bvT-f32-punned, 16 condT, 128 bo
# bf16 column offset of each jc block / the wo block inside the merged tensor
_COL = {0: SM, 1: SM + 1024, 2: SM + 2048, 3: SM + 3072, "wo": SM + 4096,
        4: SM + 5120, 5: SM + 6144, 6: SM + 7168, 7: SM + 8192}
WVO_COLS = SM + 9 * 1024  # 9376
# input chunks spread over the three DMA queues, ~balanced bytes each
_CHUNKS = [  # (engine, start, end)
    ("sync", 0, _COL[1]),                    # smalls + jc0      (296 KB)
    ("scalar", _COL["wo"], _COL[4]),         # wo                (256 KB)
    ("gpsimd", _COL[7], WVO_COLS),           # jc7               (256 KB)
    ("sync", _COL[2], _COL[3]),              # jc2               (256 KB)
    ("scalar", _COL[1], _COL[2]),            # jc1               (256 KB)
    ("sync", _COL[4], _COL[5]),              # jc4               (256 KB)
    ("scalar", _COL[3], _COL["wo"]),         # jc3               (256 KB)
    ("sync", _COL[6], _COL[7]),              # jc6               (256 KB)
    ("scalar", _COL[5], _COL[6]),            # jc5               (256 KB)
]
# stage-1 consumption order ~ expected chunk arrival order
_ORDER = [0, 7, 2, 1, 4, 3, 6, 5]

_cache = {}


def _build():
    f32 = mybir.dt.float32
    bf16 = mybir.dt.bfloat16
    nc = bacc.Bacc()

    wvo = nc.dram_tensor("wvo", [128, WVO_COLS], bf16, kind="ExternalInput")
    y = nc.dram_tensor("y", [128, B * SA * NW], bf16, kind="ExternalOutput")

    with tile.TileContext(nc) as tc, ExitStack() as ctx:
        wvo_pool = ctx.enter_context(tc.tile_pool(name="wvo", bufs=1))
        small = ctx.enter_context(tc.tile_pool(name="small", bufs=1))
        outp = ctx.enter_context(tc.tile_pool(name="outp", bufs=1))
        pools = {}
        for name in ("warm", "v1a", "v1b", "row0", "row1", "out0", "out1"):
            pools[name] = ctx.enter_context(
                tc.tile_pool(name=f"ps_{name}", bufs=1, space=bass.MemorySpace.PSUM)
            )

        from concourse.tile_rust import add_dep_helper

        # constants (no DMA needed)
        ones1 = small.tile([1, NW], bf16)
        nc.vector.memset(ones1[:], 1.0)
        warm = small.tile([128, 128], bf16)
        nc.vector.memset(warm[:], 0.0)

        # ---- loads: chunks spread across the 3 DMA queues ----
        wvo_sb = wvo_pool.tile([128, WVO_COLS], bf16)
        engines = {"sync": nc.sync, "scalar": nc.scalar, "gpsimd": nc.gpsimd}
        prev = {k: None for k in engines}
        for ring, c0, c1 in _CHUNKS:
            d = engines[ring].dma_start(wvo_sb[:, c0:c1], wvo[:, c0:c1])
            if prev[ring] is not None:
                add_dep_helper(d.ins, prev[ring].ins, sync=False, reason="ring order")
            prev[ring] = d

        bvT = wvo_sb[:, 0:16].bitcast(f32)  # [128, 8] f32
        condT = wvo_sb[:, 16:32]  # [128, 16] bf16
        bo_ap = wvo_sb[0:1, 32:160]  # [1, 128] bf16

        # ---- PE warmup: lift the HAM clock gate while DMA streams in ----
        warm_ps = pools["warm"].tile([128, 128], f32)
        for _ in range(10):
            nc.tensor.matmul(warm_ps[:], warm[:], warm[:], start=True, stop=True)

        # ---- stage 1: v1T[:, jc*2+b] = sum_k WvT[k, jc*128+p] cond[b, k] ----
        v1_ps = [pools["v1a"].tile([128, JC], f32, name="v1a_t"),
                 pools["v1b"].tile([128, JC], f32, name="v1b_t")]
        v1T_sb = small.tile([128, JC * B], bf16)
        for n, jc in enumerate(_ORDER):
            base = _COL[jc]
            acc = v1_ps[n % 2][:, (n // 2) * B : (n // 2) * B + B]
            for kc in range(KC):
                nc.tensor.matmul(
                    acc,
                    wvo_sb[:, base + kc * 128 : base + (kc + 1) * 128],
                    condT[:, kc * B : (kc + 1) * B],
                    start=(kc == 0),
                    stop=(kc == KC - 1),
                )
            # fold in bv, cast to bf16 for stage 2
            nc.vector.tensor_add(
                v1T_sb[:, jc * B : (jc + 1) * B],
                acc,
                bvT[:, jc : jc + 1].broadcast_to([128, B]),
            )

        # ---- stage 2 per batch: row[b, n] = sum_j v1[b, j] WoT[j, n] + bo ----
        wo0 = _COL["wo"]
        row_ps = [pools["row0"].tile([1, NW], f32, name="row0_t"),
                  pools["row1"].tile([1, NW], f32, name="row1_t")]
        row_sb = [small.tile([1, NW], bf16, name=f"row_sb{b}") for b in range(B)]
        out_ps = [pools["out0"].tile([128, NW], f32, name="out0_t"),
                  pools["out1"].tile([128, NW], f32, name="out1_t")]
        out_sb = outp.tile([128, B * NW], bf16)
        for b in range(B):
            for i, jc in enumerate(_ORDER):
                nc.tensor.matmul(
                    row_ps[b][:],
                    v1T_sb[:, jc * B + b : jc * B + b + 1],
                    wvo_sb[:, wo0 + jc * NW : wo0 + (jc + 1) * NW],
                    start=(i == 0),
                    stop=False,
                )
            nc.tensor.matmul(
                row_ps[b][:], ones1[0:1, 0:1], bo_ap, start=False, stop=True
            )
            if b == 0:
                nc.vector.tensor_copy(row_sb[b][:], row_ps[b][:])
            else:
                nc.scalar.activation(
                    row_sb[b][:], row_ps[b][:],
                    mybir.ActivationFunctionType.Copy,
                )
            # broadcast across partitions (seq positions)
            nc.tensor.matmul(
                out_ps[b][:], ones1[:], row_sb[b][:], start=True, stop=True
            )
            if b == 0:
                nc.vector.tensor_copy(out_sb[:, b * NW : (b + 1) * NW], out_ps[b][:])
            else:
                nc.scalar.activation(
                    out_sb[:, b * NW : (b + 1) * NW], out_ps[b][:],
                    mybir.ActivationFunctionType.Copy,
                )

        # ---- broadcast-writes (bf16), one batch per HWDGE queue ----
        def wr(eng_name, b, a0, a1):
            eng = engines[eng_name]
            d = eng.dma_start(
                y[:, b * SA * NW + a0 * NW : b * SA * NW + a1 * NW].rearrange(
                    "p (a c) -> p a c", a=a1 - a0
                ),
                out_sb[:, b * NW : (b + 1) * NW][:, None, :].broadcast_to(
                    [128, a1 - a0, NW]
                ),
            )
            add_dep_helper(d.ins, prev[eng_name].ins, sync=False, reason="ring order")

        wr("sync", 0, 0, SA)
        wr("scalar", 1, 0, SA)

    nc.compile()
    return nc


def _prep_inputs(condition, Wv, bv, Wo, bo):
    cond = np.asarray(condition, np.float32)
    Wv = np.asarray(Wv, np.float32)
    Wo = np.asarray(Wo, np.float32)
    bv = np.asarray(bv, np.float32)
    bo = np.asarray(bo, np.float32)

    # wv block jc: [p, kc*128+c] = Wv[jc*128+c, kc*128+p]
    wv_blocks = (
        Wv.reshape(JC, 128, KC, 128).transpose(3, 0, 2, 1).astype(BF16)
    )  # [128, JC, KC, 128]
    # condT[p, kc*B+b] = cond[b, kc*128+p]
    condT = np.ascontiguousarray(
        cond.T.reshape(KC, 128, B).transpose(1, 0, 2).reshape(128, KC * B)
    ).astype(BF16)
    bvT = np.ascontiguousarray(bv.reshape(JC, 128).T)  # [128, 8] f32

    smalls_base = np.zeros((128, SM), BF16)
    smalls_base[:, 0:16] = bvT.view(BF16)  # bit-punned f32
    smalls_base[:, 16:32] = condT

    in_maps = []
    for i in range(NCORES):
        # wo[p, jc*NW+c] = Wo[i*NW+c, jc*128+p]
        wo_i = np.ascontiguousarray(
            Wo[i * NW : (i + 1) * NW]
            .reshape(NW, JC, 128)
            .transpose(2, 1, 0)
            .reshape(128, JC * NW)
        ).astype(BF16)
        smalls = smalls_base.copy()
        smalls[0, 32:160] = bo[i * NW : (i + 1) * NW].astype(BF16)
        wvo = np.concatenate(
            [
                smalls,
                wv_blocks[:, 0:2].reshape(128, 2048),
                wv_blocks[:, 2:4].reshape(128, 2048),
                wo_i,
                wv_blocks[:, 4:6].reshape(128, 2048),
                wv_blocks[:, 6:8].reshape(128, 2048),
            ],
            axis=1,
        )
        in_maps.append({"wvo": np.ascontiguousarray(wvo)})
    return in_maps


def _run(in_maps, **kwargs):
    if "nc" not in _cache:
        _cache["nc"] = _build()
    return run_bass_kernel_spmd(
        _cache["nc"], in_maps, core_ids=list(range(NCORES)), **kwargs
    )


def kernel(hidden_states, condition, Wq, bq, Wk, bk, Wv, bv, Wo, bo):
    in_maps = _prep_inputs(condition, Wv, bv, Wo, bo)
    res = _run(in_maps)
    full = np.empty((B, S, D), np.float32)
    for i in range(NCORES):
        yv = np.asarray(res.results[i]["y"]).astype(np.float32).reshape(128, B, SA, NW)
        full[:, :, i * NW : (i + 1) * NW] = (
            yv.transpose(1, 2, 0, 3).reshape(B, S, NW)
        )
    return full


# revision 19
# speedup vs baseline: 1.1819x; 1.0245x over previous
"""Trainium2 Bass kernel for CrossAttentionConditionInjection.

Math: the attention keys/values come from a single condition token broadcast
across the sequence, so the scores are constant along the key axis; softmax is
exactly uniform and the attention output collapses to

    out[b, s, :] = (condition[b] @ Wv.T + bv) @ Wo.T + bo      (for every s)

independent of hidden_states / Wq / Wk / q entirely.

Sharding: core i owns output columns [128*i, 128*(i+1)) for BOTH batches.
Weights stream in bf16 (rel-err budget 2e-2 >> bf16's ~4e-3).  All inputs ride
ONE merged bf16 tensor split into ~256KB chunks spread over the THREE
DMA-capable queues (sync/scalar HWDGE + gpsimd SWDGE) so the combined read
rate saturates the per-core HBM limit; stage 1 consumes j-chunks in expected
arrival order.  smalls packs bv (f32 bit-punned), condT and bo and is
recovered with AP bitcasts.  Compute is all PE:

  stage 1: per j-chunk jc, 8 matmuls  lhsT=WvT block [128k,128j] (stationary),
           rhs=condT k-slice [128k, 2b] -> v1T_ps[.., 2]  (accum over k),
           double-buffered across two PSUM banks so the DVE bv-add/cast of
           chunk jc never blocks the matmuls of chunk jc+1 (PSUM WAR is
           tracked at bank granularity).
  stage 2: per batch b: 8 matmuls lhsT=v1T_sb jc-slice [128j, 1],
           rhs=WoT block [128j, 128n] -> row_b [1, 128n] (+ [1,1]x[1,128]
           ones-matmul folds bo), then one matmul lhsT=ones[1,128],
           rhs=row_b bf16 -> out_b [128 (s-copies), 128n].
  write:   batch 0 as one 1MB DMA on sync, batch 1 as two 512KB halves on
           scalar+gpsimd; each broadcast-writes its [128,128] row tile with
           per-partition-contiguous lines.  Batch-0's write overlaps batch-1's
           tail compute.

A few dummy matmuls head the PE queue to lift the HAM clock gate
(1.2 -> 2.4 GHz) while the first weight chunks are still in flight.
"""

import numpy as np
from contextlib import ExitStack

import ml_dtypes

import concourse.bass as bass
import concourse.bacc as bacc
import concourse.mybir as mybir
import concourse.tile as tile
from concourse.bass_utils import run_bass_kernel_spmd

B, S, D = 2, 2048, 1024
NCORES = 8
NW = D // NCORES  # 128 output columns per core
KC = D // 128  # 8 contraction chunks (k)
JC = D // 128  # 8 v1 chunks (j)
SA = S // 128  # 16 sequence blocks
BF16 = ml_dtypes.bfloat16

SM = 160  # smalls section cols (bf16): 16 bvT-f32-punned, 16 condT, 128 bo
# bf16 column offset of each jc block / the wo block inside the merged tensor
# layout: [smalls][jc0] [jc1][jc2] [wo][jc3] [jc4][jc5] [jc6] [jc7]
_COL = {0: SM, 1: SM + 1024, 2: SM + 2048, "wo": SM + 3072, 3: SM + 4096,
        4: SM + 5120, 5: SM + 6144, 6: SM + 7168, 7: SM + 8192}
WVO_COLS = SM + 9 * 1024  # 9376
# input chunks: two per DMA queue, ~balanced bytes
_CHUNKS = [  # (engine, start, end)
    ("sync", 0, _COL[1]),                    # smalls + jc0      (296 KB)
    ("scalar", _COL[1], _COL["wo"]),         # jc1 + jc2         (512 KB)
    ("gpsimd", _COL["wo"], _COL[4]),         # wo + jc3          (512 KB)
    ("sync", _COL[4], _COL[6]),              # jc4 + jc5         (512 KB)
    ("scalar", _COL[6], _COL[7]),            # jc6               (256 KB)
    ("gpsimd", _COL[7], WVO_COLS),           # jc7               (256 KB)
]
# stage-1 consumption order ~ expected chunk arrival order
_ORDER = [0, 1, 2, 3, 4, 5, 6, 7]

_cache = {}


def _build():
    f32 = mybir.dt.float32
    bf16 = mybir.dt.bfloat16
    nc = bacc.Bacc()

    wvo = nc.dram_tensor("wvo", [128, WVO_COLS], bf16, kind="ExternalInput")
    y = nc.dram_tensor("y", [128, B * SA * NW], bf16, kind="ExternalOutput")

    with tile.TileContext(nc) as tc, ExitStack() as ctx:
        wvo_pool = ctx.enter_context(tc.tile_pool(name="wvo", bufs=1))
        small = ctx.enter_context(tc.tile_pool(name="small", bufs=1))
        outp = ctx.enter_context(tc.tile_pool(name="outp", bufs=1))
        pools = {}
        for name in ("warm", "v1a", "v1b", "row0", "row1", "out0", "out1"):
            pools[name] = ctx.enter_context(
                tc.tile_pool(name=f"ps_{name}", bufs=1, space=bass.MemorySpace.PSUM)
            )

        from concourse.tile_rust import add_dep_helper

        # constants (no DMA needed)
        ones1 = small.tile([1, NW], bf16)
        nc.vector.memset(ones1[:], 1.0)
        warm = small.tile([128, 128], bf16)
        nc.vector.memset(warm[:], 0.0)

        # ---- loads: chunks spread across the 3 DMA queues ----
        wvo_sb = wvo_pool.tile([128, WVO_COLS], bf16)
        engines = {"sync": nc.sync, "scalar": nc.scalar, "gpsimd": nc.gpsimd}
        prev = {k: None for k in engines}
        for ring, c0, c1 in _CHUNKS:
            d = engines[ring].dma_start(wvo_sb[:, c0:c1], wvo[:, c0:c1])
            if prev[ring] is not None:
                add_dep_helper(d.ins, prev[ring].ins, sync=False, reason="ring order")
            prev[ring] = d

        bvT = wvo_sb[:, 0:16].bitcast(f32)  # [128, 8] f32
        condT = wvo_sb[:, 16:32]  # [128, 16] bf16
        bo_ap = wvo_sb[0:1, 32:160]  # [1, 128] bf16

        # ---- PE warmup: lift the HAM clock gate while DMA streams in ----
        warm_ps = pools["warm"].tile([128, 128], f32)
        for _ in range(10):
            nc.tensor.matmul(warm_ps[:], warm[:], warm[:], start=True, stop=True)

        # ---- stage 1: v1T[:, jc*2+b] = sum_k WvT[k, jc*128+p] cond[b, k] ----
        v1_ps = [pools["v1a"].tile([128, JC], f32, name="v1a_t"),
                 pools["v1b"].tile([128, JC], f32, name="v1b_t")]
        v1T_sb = small.tile([128, JC * B], bf16)
        for n, jc in enumerate(_ORDER):
            base = _COL[jc]
            acc = v1_ps[n % 2][:, (n // 2) * B : (n // 2) * B + B]
            for kc in range(KC):
                nc.tensor.matmul(
                    acc,
                    wvo_sb[:, base + kc * 128 : base + (kc + 1) * 128],
                    condT[:, kc * B : (kc + 1) * B],
                    start=(kc == 0),
                    stop=(kc == KC - 1),
                )
            # fold in bv, cast to bf16 for stage 2
            nc.vector.tensor_add(
                v1T_sb[:, jc * B : (jc + 1) * B],
                acc,
                bvT[:, jc : jc + 1].broadcast_to([128, B]),
            )
            if n == 3:
                # keep the HAM clock gate open across the mid-stream DMA gap
                for _ in range(8):
                    nc.tensor.matmul(
                        warm_ps[:], warm[:], warm[:], start=True, stop=True
                    )

        # ---- stage 2 per batch: row[b, n] = sum_j v1[b, j] WoT[j, n] + bo ----
        wo0 = _COL["wo"]
        row_ps = [pools["row0"].tile([1, NW], f32, name="row0_t"),
                  pools["row1"].tile([1, NW], f32, name="row1_t")]
        row_sb = [small.tile([1, NW], bf16, name=f"row_sb{b}") for b in range(B)]
        out_ps = [pools["out0"].tile([128, NW], f32, name="out0_t"),
                  pools["out1"].tile([128, NW], f32, name="out1_t")]
        out_sb = outp.tile([128, B * NW], bf16)
        for b in range(B):
            for i, jc in enumerate(_ORDER):
                nc.tensor.matmul(
                    row_ps[b][:],
                    v1T_sb[:, jc * B + b : jc * B + b + 1],
                    wvo_sb[:, wo0 + jc * NW : wo0 + (jc + 1) * NW],
                    start=(i == 0),
                    stop=False,
                )
            nc.tensor.matmul(
                row_ps[b][:], ones1[0:1, 0:1], bo_ap, start=False, stop=True
            )
            if b == 0:
                nc.vector.tensor_copy(row_sb[b][:], row_ps[b][:])
            else:
                nc.scalar.activation(
                    row_sb[b][:], row_ps[b][:],
                    mybir.ActivationFunctionType.Copy,
                )
            # broadcast across partitions (seq positions)
            nc.tensor.matmul(
                out_ps[b][:], ones1[:], row_sb[b][:], start=True, stop=True
            )
            if b == 0:
                nc.vector.tensor_copy(out_sb[:, b * NW : (b + 1) * NW], out_ps[b][:])
            else:
                nc.scalar.activation(
                    out_sb[:, b * NW : (b + 1) * NW], out_ps[b][:],
                    mybir.ActivationFunctionType.Copy,
                )

        # ---- broadcast-writes (bf16), one batch per HWDGE queue ----
        def wr(eng_name, b, a0, a1):
            eng = engines[eng_name]
            d = eng.dma_start(
                y[:, b * SA * NW + a0 * NW : b * SA * NW + a1 * NW].rearrange(
                    "p (a c) -> p a c", a=a1 - a0
                ),
                out_sb[:, b * NW : (b + 1) * NW][:, None, :].broadcast_to(
                    [128, a1 - a0, NW]
                ),
            )
            add_dep_helper(d.ins, prev[eng_name].ins, sync=False, reason="ring order")

        wr("sync", 0, 0, SA)
        wr("scalar", 1, 0, SA)

    nc.compile()
    return nc


def _prep_inputs(condition, Wv, bv, Wo, bo):
    cond = np.asarray(condition, np.float32)
    Wv = np.asarray(Wv, np.float32)
    Wo = np.asarray(Wo, np.float32)
    bv = np.asarray(bv, np.float32)
    bo = np.asarray(bo, np.float32)

    # wv block jc: [p, kc*128+c] = Wv[jc*128+c, kc*128+p]
    wv_blocks = (
        Wv.reshape(JC, 128, KC, 128).transpose(3, 0, 2, 1).astype(BF16)
    )  # [128, JC, KC, 128]
    # condT[p, kc*B+b] = cond[b, kc*128+p]
    condT = np.ascontiguousarray(
        cond.T.reshape(KC, 128, B).transpose(1, 0, 2).reshape(128, KC * B)
    ).astype(BF16)
    bvT = np.ascontiguousarray(bv.reshape(JC, 128).T)  # [128, 8] f32

    smalls_base = np.zeros((128, SM), BF16)
    smalls_base[:, 0:16] = bvT.view(BF16)  # bit-punned f32
    smalls_base[:, 16:32] = condT

    in_maps = []
    for i in range(NCORES):
        # wo[p, jc*NW+c] = Wo[i*NW+c, jc*128+p]
        wo_i = np.ascontiguousarray(
            Wo[i * NW : (i + 1) * NW]
            .reshape(NW, JC, 128)
            .transpose(2, 1, 0)
            .reshape(128, JC * NW)
        ).astype(BF16)
        smalls = smalls_base.copy()
        smalls[0, 32:160] = bo[i * NW : (i + 1) * NW].astype(BF16)
        wvo = np.concatenate(
            [
                smalls,
                wv_blocks[:, 0:3].reshape(128, 3072),
                wo_i,
                wv_blocks[:, 3:8].reshape(128, 5120),
            ],
            axis=1,
        )
        in_maps.append({"wvo": np.ascontiguousarray(wvo)})
    return in_maps


def _run(in_maps, **kwargs):
    if "nc" not in _cache:
        _cache["nc"] = _build()
    return run_bass_kernel_spmd(
        _cache["nc"], in_maps, core_ids=list(range(NCORES)), **kwargs
    )


def kernel(hidden_states, condition, Wq, bq, Wk, bk, Wv, bv, Wo, bo):
    in_maps = _prep_inputs(condition, Wv, bv, Wo, bo)
    res = _run(in_maps)
    full = np.empty((B, S, D), np.float32)
    for i in range(NCORES):
        yv = np.asarray(res.results[i]["y"]).astype(np.float32).reshape(128, B, SA, NW)
        full[:, :, i * NW : (i + 1) * NW] = (
            yv.transpose(1, 2, 0, 3).reshape(B, S, NW)
        )
    return full
